# revision 18
# baseline (speedup 1.0000x reference)
"""Trainium2 Bass kernel for nn_HeatmapLayer: separable Gaussian heatmaps.

Reference math (per batch b, class c):
    mx = labels[b, 2c] * H ; my = labels[b, 2c+1] * W          (H = W = 384)
    sigma = H * exp(log_weight)
    dx2[h] = (h - mx)^2 / sigma        ; normalized by its min over h
    dy2[w] = (w - my)^2 / (20 * sigma) ; normalized by its min over w
    out[b,c,h,w] = exp(-0.5*(dx2[h] + dy2[w])) = ex[h] * ey[w]

Each (b,c) heatmap is a rank-1 outer product of two 384-length
profiles.  Pure data parallel over batch: 12 (b,c) pairs per core;
per-core output 7.08 MB, HBM-write roofline ~19.8us at 358 GB/s.
The kernel is latency-shaped: a fixed ~6.5us NEFF prologue + input
DMA completion gate the first output DMA; the write stream then
runs at the HBM cap (~18.2us span) with a ~2.5us completion tail.
Optimization = start the stream ASAP.

v8 structure (v5 39.2us, v7 38.7us measured):

  * Inputs enter with MINIMAL descriptor counts (completion latency
    scales with descriptor count: 1 desc ~1.1us, 12 descs ~2.5us):
    log_weight as [1,1] (1 desc), labels as [2,12] (2 descs).  The
    PE broadcasts lw to 12 partitions (ones^T @ lw) and transposes
    labels - both into one small PSUM tile, so no many-descriptor
    input DMA ever sits on the critical path.
  * q-layout profile math: partition q = 2c+i holds side i (x/y) of
    class c, batch b in the free dim.  The per-side sqrt-scale
    ssq_q = exp(-logw/2 + bias_q) needs a parity-dependent bias,
    built early from an int iota + shift (no input deps).  One
    fused DVE op turns the PSUM label transpose into both Square
    biases: sb = (labT * ssq_q) * -H.  Per batch: ACT Square (scale
    folded), DVE min-reduce, ACT Exp (min as bias) - all 12x384.
  * Row 2c+1 of the profile block IS ey_(b,c): the otherwise-idle
    PE replicates it to 128 partitions with a bf16 selection-matrix
    matmul (sel_k[m] = (k==2c+1), 1 cycle/row; ~2e-3 rel err vs the
    2e-2 gate).  Row 2c feeds the x-transposes: 3 strided [12,128]
    PE transposes per batch -> ext[par, r, b, q] = e[q, b, 3par+r].
  * 3 chunk multiplies per pair read the ey PSUM directly (DVE/ACT
    split), one 576KB DMA per pair on the sync HWDGE queue (4608B
    per-partition descriptors; pair 0 split 1+2 chunks to start the
    stream earliest).
  * A dummy ACT op forces the 1283ns Exp-table load into the
    prologue window.

x is only used for its shape; it is never transferred to the device.
"""

import numpy as np
from contextlib import ExitStack

import concourse.bacc as bacc
import concourse.bass as bass
import concourse.tile as tile
from concourse import mybir
from concourse.bass_utils import run_bass_kernel_spmd
from concourse.masks import make_identity

B, CH, H, W = 16, 3, 384, 384
NCLS = 6
N_CORES = 8
BPC = B // N_CORES            # batches per core = 2
PAIRS = BPC * NCLS            # (b,c) pairs per core = 12
Q = 2 * NCLS                  # profile rows per batch (q = 2c+i) = 12
P = 128
RPP = H // P                  # DRAM rows per partition = 3
LN_H = float(np.log(H))
F32 = mybir.dt.float32
BF16 = mybir.dt.bfloat16
I32 = mybir.dt.int32
AF = mybir.ActivationFunctionType

# ssq_q = sqrt(k_i*inv_s) = exp(-0.5*logw + BIAS_SX + (q%2)*LN_RATIO)
BIAS_SX = -LN_H / 2 + float(np.log(0.5)) / 2     # x side: k = 1/2
LN_RATIO = 0.5 * float(np.log(0.05))             # +ln(ssq_y/ssq_x)

# engine for the 36 final multiplies, by flat index (p*RPP + r)
MULT_ENGINE = "vsv" + "vvs" * (PAIRS - 1)


def build_bass() -> bass.Bass:
    nc = bacc.Bacc("TRN2", target_bir_lowering=False, debug=False,
                   num_devices=N_CORES)
    labels = nc.dram_tensor("labels", [BPC, Q], F32, kind="ExternalInput")
    logw = nc.dram_tensor("log_weight", [1, 1], F32, kind="ExternalInput")
    out = nc.dram_tensor("out", [PAIRS * H, W], F32, kind="ExternalOutput")

    with ExitStack() as ctx:
        tc = ctx.enter_context(tile.TileContext(nc))
        singles = ctx.enter_context(tc.tile_pool(name="singles", bufs=1))
        psum = ctx.enter_context(tc.tile_pool(name="psum", bufs=6,
                                              space="PSUM"))
        psumT = ctx.enter_context(tc.tile_pool(name="psumT", bufs=1,
                                               space="PSUM"))
        psumS = ctx.enter_context(tc.tile_pool(name="psumS", bufs=1,
                                               space="PSUM"))
        stage = ctx.enter_context(tc.tile_pool(name="stage", bufs=8))

        # ---- constants (no input deps; overlap prologue/input DMAs).
        # gpsimd order = urgency: idents (feed bias_q + PE transposes),
        # iog (feeds Squares ~9.9us), sel (feeds matmuls ~12.7us).
        ones = singles.tile([1, Q], F32)
        nc.vector.memset(ones, 1.0)
        ident2 = singles.tile([BPC, BPC], F32)
        make_identity(nc, ident2)
        ident = singles.tile([Q, Q], F32)
        make_identity(nc, ident)
        iog = singles.tile([Q, W], F32)
        nc.gpsimd.iota(iog, pattern=[[1, W]], base=0, channel_multiplier=0,
                       allow_small_or_imprecise_dtypes=True)
        # sel[k, j, m] = 1.0 if k == j else 0.0  (bf16 PE broadcast weights)
        sel = singles.tile([Q, Q, P], BF16)
        nc.gpsimd.memset(sel, 1.0)
        nc.gpsimd.affine_select(
            out=sel, in_=sel, compare_op=mybir.AluOpType.is_equal,
            fill=0.0, base=0, channel_multiplier=1,
            pattern=[[-1, Q], [0, P]],
        )
        # bias_q = BIAS_SX + (q%2)*LN_RATIO; parity(q) = sum of ident's
        # odd columns in row q (ident is ready ~150ns into the kernel)
        parity = singles.tile([Q, 1], F32)
        idv = ident[:, :].rearrange("q (c two) -> q two c", two=2)
        nc.vector.tensor_reduce(out=parity, in_=idv[:, 1, :],
                                axis=mybir.AxisListType.X,
                                op=mybir.AluOpType.add)
        bias_q = singles.tile([Q, 1], F32)
        nc.vector.tensor_scalar(out=bias_q, in0=parity, scalar1=LN_RATIO,
                                scalar2=BIAS_SX, op0=mybir.AluOpType.mult,
                                op1=mybir.AluOpType.add)
        # dummy ACT op: forces the 1283ns Exp-table load to run early
        warm = singles.tile([1, 1], F32)
        nc.scalar.activation(out=warm, in_=ones[:, 0:1], func=AF.Exp,
                             bias=0.0, scale=0.0)

        # ---- inputs: minimal descriptor counts, one per HWDGE queue so
        # both issue at t~7.2us in parallel instead of serializing ------
        lw0 = singles.tile([1, 1], F32)
        nc.sync.dma_start(out=lw0, in_=logw[:, :])
        lab2 = singles.tile([BPC, Q], F32)
        nc.scalar.dma_start(out=lab2, in_=labels[:, :])

        # ---- PE: lw broadcast to [12,1], labels transposed to [12,2] ----
        # pin[q, 0] = logw; pin[q, 1+b] = labels[b, q]
        pin = psumS.tile([Q, 1 + BPC], F32)
        nc.tensor.matmul(pin[:, 0:1], ones[:, :], lw0[:, :],
                         start=True, stop=True)
        nc.tensor.transpose(pin[:, 1:1 + BPC], lab2[:, :], ident2)

        # ---- ssq_q[12,1]; Square biases sb[q, b] = -ssq_q*m[q, b] -------
        ssq = singles.tile([Q, 1], F32)
        nc.scalar.activation(out=ssq, in_=pin[:, 0:1], func=AF.Exp,
                             bias=bias_q, scale=-0.5)
        sb = singles.tile([Q, BPC], F32)
        nc.vector.tensor_scalar(out=sb, in0=pin[:, 1:1 + BPC], scalar1=ssq,
                                scalar2=-float(H),
                                op0=mybir.AluOpType.mult,
                                op1=mybir.AluOpType.mult)

        # ---- profiles per batch: sq = (ssq*(w-m))^2 ; e = exp(mn-sq) ----
        sq = singles.tile([Q, BPC, W], F32)
        mn = singles.tile([Q, BPC], F32)
        e = singles.tile([Q, BPC, W], F32)
        eb = singles.tile([Q, BPC, W], BF16)
        for b in range(BPC):
            nc.scalar.activation(out=sq[:, b, :], in_=iog, func=AF.Square,
                                 bias=sb[:, b:b + 1], scale=ssq)
            nc.vector.tensor_reduce(out=mn[:, b:b + 1], in_=sq[:, b, :],
                                    axis=mybir.AxisListType.X,
                                    op=mybir.AluOpType.min)
            nc.scalar.activation(out=e[:, b, :], in_=sq[:, b, :],
                                 func=AF.Exp, bias=mn[:, b:b + 1],
                                 scale=-1.0)
            nc.vector.tensor_copy(out=eb[:, b, :], in_=e[:, b, :])

        # ---- x-transposes + main loop, batch-major ----------------------
        # ext[par, r, b, q] = e[q, b, 3*par + r]  (only even q consumed)
        pt = psumT.tile([P, RPP, BPC, Q], F32)
        ext = singles.tile([P, RPP, BPC, Q], F32)
        for b in range(BPC):
            ev = e[:, b, :].rearrange("q (k r) -> q r k", r=RPP)
            for r in range(RPP):
                nc.tensor.transpose(pt[:, r, b, :], ev[:, r, :], ident)
            nc.vector.tensor_copy(out=ext[:, :, b, :], in_=pt[:, :, b, :])
            for c in range(NCLS):
                p = b * NCLS + c
                ps = psum.tile([P, W], F32)
                nc.tensor.matmul(ps, sel[:, 2 * c + 1, :], eb[:, b, :],
                                 start=True, stop=True)
                st = stage.tile([P, RPP, W], F32)
                for r in range(RPP):
                    scal = ext[:, r, b, 2 * c:2 * c + 1]
                    if MULT_ENGINE[p * RPP + r] == "v":
                        nc.vector.tensor_scalar_mul(out=st[:, r, :],
                                                    in0=ps, scalar1=scal)
                    else:
                        nc.scalar.mul(out=st[:, r, :], in_=ps, mul=scal)
                # partition par holds DRAM rows 3*par..3*par+2 of pair p:
                # one contiguous 4608B descriptor per partition.
                odst = out[p * H:(p + 1) * H, :].rearrange(
                    "(par r) w -> par r w", par=P)
                # alternate the two HWDGE queues: each SDMA engine then
                # drains two rings at packet granularity, which spreads
                # the slow-engine-15 load across both queue rows
                dma_eng = nc.sync if p % 2 == 0 else nc.scalar
                if p == 0:
                    # split: start the stream as soon as chunk 0 exists
                    dma_eng.dma_start(out=odst[:, 0:1, :],
                                      in_=st[:, 0:1, :])
                    dma_eng.dma_start(out=odst[:, 1:, :], in_=st[:, 1:, :])
                else:
                    dma_eng.dma_start(out=odst, in_=st)
    nc.finalize()
    return nc


LAST_RESULTS = None  # BassKernelResults of the most recent kernel() call


def kernel(x: np.ndarray, labels: np.ndarray,
           log_weight: np.ndarray, **run_kwargs) -> np.ndarray:
    global LAST_RESULTS
    del x  # only its (hardcoded) shape matters
    nc = build_bass()
    labels = np.ascontiguousarray(labels, dtype=np.float32)
    lw = np.ascontiguousarray(log_weight, dtype=np.float32).reshape(1, 1)
    in_maps = [
        {"labels": labels[i * BPC:(i + 1) * BPC], "log_weight": lw}
        for i in range(N_CORES)
    ]
    res = run_bass_kernel_spmd(nc, in_maps, core_ids=list(range(N_CORES)),
                               **run_kwargs)
    LAST_RESULTS = res
    outs = [r["out"].reshape(BPC, NCLS, H, W) for r in res.results]
    return np.concatenate(outs, axis=0)


if __name__ == "__main__":
    rng = np.random.default_rng(0)
    x = rng.standard_normal((B, CH, H, W), dtype=np.float32)
    labels = rng.random((B, 2 * NCLS), dtype=np.float32)
    lw = rng.random((1, 1, 1, 1), dtype=np.float32)
    y = kernel(x=x, labels=labels, log_weight=lw)
    print(y.shape, y.dtype, y.min(), y.max())


# revision 19
# speedup vs baseline: 1.1005x; 1.1005x over previous
"""Trainium2 Bass kernel for nn_HeatmapLayer: separable Gaussian heatmaps.

Reference math (per batch b, class c):
    mx = labels[b, 2c] * H ; my = labels[b, 2c+1] * W          (H = W = 384)
    sigma = H * exp(log_weight)
    dx2[h] = (h - mx)^2 / sigma        ; normalized by its min over h
    dy2[w] = (w - my)^2 / (20 * sigma) ; normalized by its min over w
    out[b,c,h,w] = exp(-0.5*(dx2[h] + dy2[w])) = ex[h] * ey[w]

Each (b,c) heatmap is a rank-1 outer product of two 384-length
profiles.  Pure data parallel over batch: 12 (b,c) pairs per core;
per-core output 7.08 MB, HBM-write roofline ~19.8us at 358 GB/s.
The kernel is latency-shaped: a fixed ~6.5us NEFF prologue + input
DMA completion gate the first output DMA; the write stream then
runs at the HBM cap (~18.2us span) with a ~2.5us completion tail.
Optimization = start the stream ASAP.

v8 structure (v5 39.2us, v7 38.7us measured):

  * Inputs enter with MINIMAL descriptor counts (completion latency
    scales with descriptor count: 1 desc ~1.1us, 12 descs ~2.5us):
    log_weight as [1,1] (1 desc), labels as [2,12] (2 descs).  The
    PE broadcasts lw to 12 partitions (ones^T @ lw) and transposes
    labels - both into one small PSUM tile, so no many-descriptor
    input DMA ever sits on the critical path.
  * q-layout profile math: partition q = 2c+i holds side i (x/y) of
    class c, batch b in the free dim.  The per-side sqrt-scale
    ssq_q = exp(-logw/2 + bias_q) needs a parity-dependent bias,
    built early from an int iota + shift (no input deps).  One
    fused DVE op turns the PSUM label transpose into both Square
    biases: sb = (labT * ssq_q) * -H.  Per batch: ACT Square (scale
    folded), DVE min-reduce, ACT Exp (min as bias) - all 12x384.
  * Row 2c+1 of the profile block IS ey_(b,c): the otherwise-idle
    PE replicates it to 128 partitions with a bf16 selection-matrix
    matmul (sel_k[m] = (k==2c+1), 1 cycle/row; ~2e-3 rel err vs the
    2e-2 gate).  Row 2c feeds the x-transposes: 3 strided [12,128]
    PE transposes per batch -> ext[par, r, b, q] = e[q, b, 3par+r].
  * 3 chunk multiplies per pair read the ey PSUM directly (DVE/ACT
    split), one 576KB DMA per pair on the sync HWDGE queue (4608B
    per-partition descriptors; pair 0 split 1+2 chunks to start the
    stream earliest).
  * A dummy ACT op forces the 1283ns Exp-table load into the
    prologue window.

x is only used for its shape; it is never transferred to the device.
"""

import numpy as np
from contextlib import ExitStack

import concourse.bacc as bacc
import concourse.bass as bass
import concourse.tile as tile
from concourse import mybir
from concourse.bass_utils import run_bass_kernel_spmd
from concourse.masks import make_identity

B, CH, H, W = 16, 3, 384, 384
NCLS = 6
N_CORES = 8
BPC = B // N_CORES            # batches per core = 2
PAIRS = BPC * NCLS            # (b,c) pairs per core = 12
Q = 2 * NCLS                  # profile rows per batch (q = 2c+i) = 12
P = 128
RPP = H // P                  # DRAM rows per partition = 3
LN_H = float(np.log(H))
F32 = mybir.dt.float32
BF16 = mybir.dt.bfloat16
I32 = mybir.dt.int32
AF = mybir.ActivationFunctionType

# ssq_q = sqrt(k_i*inv_s) = exp(-0.5*logw + BIAS_SX + (q%2)*LN_RATIO)
BIAS_SX = -LN_H / 2 + float(np.log(0.5)) / 2     # x side: k = 1/2
LN_RATIO = 0.5 * float(np.log(0.05))             # +ln(ssq_y/ssq_x)

# engine for the 36 final multiplies, by flat index (p*RPP + r)
MULT_ENGINE = "vsv" + "vvs" * (PAIRS - 1)


def build_bass() -> bass.Bass:
    nc = bacc.Bacc("TRN2", target_bir_lowering=False, debug=False,
                   num_devices=N_CORES)
    labels = nc.dram_tensor("labels", [BPC, Q], F32, kind="ExternalInput")
    logw = nc.dram_tensor("log_weight", [1, 1], F32, kind="ExternalInput")
    out = nc.dram_tensor("out", [PAIRS * H, W], F32, kind="ExternalOutput")

    with ExitStack() as ctx:
        tc = ctx.enter_context(tile.TileContext(nc))
        singles = ctx.enter_context(tc.tile_pool(name="singles", bufs=1))
        psum = ctx.enter_context(tc.tile_pool(name="psum", bufs=6,
                                              space="PSUM"))
        psumT = ctx.enter_context(tc.tile_pool(name="psumT", bufs=1,
                                               space="PSUM"))
        psumS = ctx.enter_context(tc.tile_pool(name="psumS", bufs=1,
                                               space="PSUM"))
        stage = ctx.enter_context(tc.tile_pool(name="stage", bufs=8))

        # ---- constants (no input deps; overlap prologue/input DMAs).
        # gpsimd order = urgency: idents (feed bias_q + PE transposes),
        # iog (feeds Squares ~9.9us), sel (feeds matmuls ~12.7us).
        ones = singles.tile([1, Q], F32)
        nc.vector.memset(ones, 1.0)
        ident2 = singles.tile([BPC, BPC], F32)
        make_identity(nc, ident2)
        ident = singles.tile([Q, Q], F32)
        make_identity(nc, ident)
        iog = singles.tile([Q, W], F32)
        nc.gpsimd.iota(iog, pattern=[[1, W]], base=0, channel_multiplier=0,
                       allow_small_or_imprecise_dtypes=True)
        # sel[k, j, m] = 1.0 if k == j else 0.0  (bf16 PE broadcast weights)
        sel = singles.tile([Q, Q, P], BF16)
        nc.gpsimd.memset(sel, 1.0)
        nc.gpsimd.affine_select(
            out=sel, in_=sel, compare_op=mybir.AluOpType.is_equal,
            fill=0.0, base=0, channel_multiplier=1,
            pattern=[[-1, Q], [0, P]],
        )
        # bias_q = BIAS_SX + (q%2)*LN_RATIO; parity(q) = sum of ident's
        # odd columns in row q (ident is ready ~150ns into the kernel)
        parity = singles.tile([Q, 1], F32)
        idv = ident[:, :].rearrange("q (c two) -> q two c", two=2)
        nc.vector.tensor_reduce(out=parity, in_=idv[:, 1, :],
                                axis=mybir.AxisListType.X,
                                op=mybir.AluOpType.add)
        bias_q = singles.tile([Q, 1], F32)
        nc.vector.tensor_scalar(out=bias_q, in0=parity, scalar1=LN_RATIO,
                                scalar2=BIAS_SX, op0=mybir.AluOpType.mult,
                                op1=mybir.AluOpType.add)
        # ---- inputs: minimal descriptor counts, one per HWDGE queue so
        # both issue at t~7.2us in parallel instead of serializing.
        # Emitted BEFORE any ACT compute so the lab2 DIRECT2D is the ACT
        # sequencer's first instruction (ahead of the 1283ns table load).
        lw0 = singles.tile([1, 1], F32)
        nc.sync.dma_start(out=lw0, in_=logw[:, :])
        lab2 = singles.tile([BPC, Q], F32)
        nc.scalar.dma_start(out=lab2, in_=labels[:, :])

        # dummy ACT op: forces the 1283ns Exp-table load to run early
        warm = singles.tile([1, 1], F32)
        nc.scalar.activation(out=warm, in_=ones[:, 0:1], func=AF.Exp,
                             bias=0.0, scale=0.0)

        # ---- PE: lw broadcast to [12,1], labels transposed to [12,2] ----
        # pin[q, 0] = logw; pin[q, 1+b] = labels[b, q]
        pin = psumS.tile([Q, 1 + BPC], F32)
        nc.tensor.matmul(pin[:, 0:1], ones[:, :], lw0[:, :],
                         start=True, stop=True)
        nc.tensor.transpose(pin[:, 1:1 + BPC], lab2[:, :], ident2)

        # ---- ssq_q[12,1]; Square biases sb[q, b] = -ssq_q*m[q, b] -------
        ssq = singles.tile([Q, 1], F32)
        nc.scalar.activation(out=ssq, in_=pin[:, 0:1], func=AF.Exp,
                             bias=bias_q, scale=-0.5)
        sb = singles.tile([Q, BPC], F32)
        nc.vector.tensor_scalar(out=sb, in0=pin[:, 1:1 + BPC], scalar1=ssq,
                                scalar2=-float(H),
                                op0=mybir.AluOpType.mult,
                                op1=mybir.AluOpType.mult)

        # ---- profiles per batch: sq = (ssq*(w-m))^2 ; e = exp(mn-sq) ----
        sq = singles.tile([Q, BPC, W], F32)
        mn = singles.tile([Q, BPC], F32)
        e = singles.tile([Q, BPC, W], F32)
        eb = singles.tile([Q, BPC, W], BF16)
        for b in range(BPC):
            nc.scalar.activation(out=sq[:, b, :], in_=iog, func=AF.Square,
                                 bias=sb[:, b:b + 1], scale=ssq)
            nc.vector.tensor_reduce(out=mn[:, b:b + 1], in_=sq[:, b, :],
                                    axis=mybir.AxisListType.X,
                                    op=mybir.AluOpType.min)
            nc.scalar.activation(out=e[:, b, :], in_=sq[:, b, :],
                                 func=AF.Exp, bias=mn[:, b:b + 1],
                                 scale=-1.0)
            nc.vector.tensor_copy(out=eb[:, b, :], in_=e[:, b, :])

        # ---- x-transposes + main loop, batch-major ----------------------
        # ext[par, r, b, q] = e[q, b, 3*par + r]  (only even q consumed)
        pt = psumT.tile([P, RPP, BPC, Q], F32)
        ext = singles.tile([P, RPP, BPC, Q], F32)
        for b in range(BPC):
            ev = e[:, b, :].rearrange("q (k r) -> q r k", r=RPP)
            for r in range(RPP):
                nc.tensor.transpose(pt[:, r, b, :], ev[:, r, :], ident)
            nc.vector.tensor_copy(out=ext[:, :, b, :], in_=pt[:, :, b, :])
            for c in range(NCLS):
                p = b * NCLS + c
                ps = psum.tile([P, W], F32)
                nc.tensor.matmul(ps, sel[:, 2 * c + 1, :], eb[:, b, :],
                                 start=True, stop=True)
                st = stage.tile([P, RPP, W], F32)
                for r in range(RPP):
                    scal = ext[:, r, b, 2 * c:2 * c + 1]
                    if MULT_ENGINE[p * RPP + r] == "v":
                        nc.vector.tensor_scalar_mul(out=st[:, r, :],
                                                    in0=ps, scalar1=scal)
                    else:
                        nc.scalar.mul(out=st[:, r, :], in_=ps, mul=scal)
                # partition par holds DRAM rows 3*par..3*par+2 of pair p:
                # one contiguous 4608B descriptor per partition.
                odst = out[p * H:(p + 1) * H, :].rearrange(
                    "(par r) w -> par r w", par=P)
                # alternate the two HWDGE queues: each SDMA engine then
                # drains two rings at packet granularity, which spreads
                # the slow-engine-15 load across both queue rows
                dma_eng = nc.sync if p % 2 == 0 else nc.scalar
                if p == 0:
                    # split: start the stream as soon as chunk 0 exists
                    dma_eng.dma_start(out=odst[:, 0:1, :],
                                      in_=st[:, 0:1, :])
                    dma_eng.dma_start(out=odst[:, 1:, :], in_=st[:, 1:, :])
                else:
                    dma_eng.dma_start(out=odst, in_=st)
    nc.finalize()
    return nc


LAST_RESULTS = None  # BassKernelResults of the most recent kernel() call


def kernel(x: np.ndarray, labels: np.ndarray,
           log_weight: np.ndarray, **run_kwargs) -> np.ndarray:
    global LAST_RESULTS
    del x  # only its (hardcoded) shape matters
    nc = build_bass()
    labels = np.ascontiguousarray(labels, dtype=np.float32)
    lw = np.ascontiguousarray(log_weight, dtype=np.float32).reshape(1, 1)
    in_maps = [
        {"labels": labels[i * BPC:(i + 1) * BPC], "log_weight": lw}
        for i in range(N_CORES)
    ]
    res = run_bass_kernel_spmd(nc, in_maps, core_ids=list(range(N_CORES)),
                               **run_kwargs)
    LAST_RESULTS = res
    outs = [r["out"].reshape(BPC, NCLS, H, W) for r in res.results]
    return np.concatenate(outs, axis=0)


if __name__ == "__main__":
    rng = np.random.default_rng(0)
    x = rng.standard_normal((B, CH, H, W), dtype=np.float32)
    labels = rng.random((B, 2 * NCLS), dtype=np.float32)
    lw = rng.random((1, 1, 1, 1), dtype=np.float32)
    y = kernel(x=x, labels=labels, log_weight=lw)
    print(y.shape, y.dtype, y.min(), y.max())
